# revision 77
# baseline (speedup 1.0000x reference)
"""Distributed kNN OOD-score kernel for 8 Trainium2 NeuronCores.

Problem: for each of 4*32*32 query vectors (D=768), find the 3 nearest
database vectors (N=20000, squared-L2), average the 3 distances, and
bilinearly upsample the resulting [4,32,32] map to [4,1,512,512].

Sharding: queries are data-parallel. Each core owns half of one batch
image (16 of 32 query rows = 512 queries); the database is replicated
and streamed through SBUF in fp8 (e4m3). The one halo row each core
needs for the 16x bilinear upsample is exchanged with its pair core via
a tiny AllGather, scheduled early (boundary query tile first, its last
two super-chunks processed ahead of the other tiles) so its ~15us
latency hides under the remaining compute stream.

Per-core device program (v4, fp8 DoubleRow + full PE-side max-fold):
  - every database column PAIR (x0, x1) is pre-folded on the host into
    a=(x0+x1)/2 and b=(x0-x1)/2 streams. The device computes
        u = q.a + xh_a   and   v = q.b + xh_b
    (so u,v = (t0+-t1)/2 for scores t = q.x - ||x||^2/2), ScalarE takes
    |v| to SBUF bf16, and TensorE adds it onto u with a bf16 identity
    matmul: u + |v| = max(t0, t1) exactly. VectorE then scans only
    1000 folded values per 2000-col half-strip.
  - the cross products keep 764 of 768 dims; the 4 freed contraction
    rows carry xh_* as a 4-level fp8 split (x4 stationary scale baked
    into constant query rows), so scoring is EXACTLY 3 fp8 e4m3
    DoubleRow matmuls per bank (0.5 cyc/col each). ||q||^2 and ||x||^2
    stay exact over all 768 dims; the dropped cross terms add ~4 rms
    on d^2 ~ 1536 (top-3 spacing ~20) -- well inside the tolerance.
  - per query-tile: max8 over the 10 strip top-8s -> top-3, then
    mean distance = reduce_sum of sqrt((q^2 - 2t)/9); ood values are
    transposed into map layout with 4 tiny selector matmuls on the PE
    (no DRAM round-trip).
  - pair AllGather of the boundary tile's 128 ood values (bf16).
  - 16x bilinear upsample = two small bf16 matmuls (interp weights are
    odd/32 fractions - exact in bf16; verified vs jax.image.resize).
"""

import sys

if "/opt/trn_rl_repo" not in sys.path:
    sys.path.insert(0, "/opt/trn_rl_repo")

import numpy as np
import ml_dtypes

import concourse.bass as bass
import concourse.bacc as bacc
import concourse.mybir as mybir
import concourse.tile as tile
from concourse import bass_utils

# Problem shape (hardcoded per contract).
B, D, H, W = 4, 768, 32, 32
N = 20000
K_NN = 3
OUT_H = OUT_W = 512
N_CORES = 8

SC = 4000            # db columns per super-chunk
N_SC = N // SC       # 5
N_HS = 2 * N_SC      # 2000-col half-strips per query tile
QPC = 512            # queries scored per core (16 rows)
N_QT = QPC // 128    # 4
NKP = 3              # K pairs: 764 data dims + 4 xh rows = 3 * (2*128)
DX = 764             # cross-term dims (768 minus 4 freed for xh rows)
OROWS = 256          # output rows per core
NCOL = 24            # ood columns entering the upsample (16 own + 2x4 gathered)
XS = 4.0             # xh scale, baked into constant query rows (fp8 range fit)
DEBUG = False        # adds intermediate-tensor outputs for debugging

F32 = mybir.dt.float32
BF16 = mybir.dt.bfloat16
FP8 = mybir.dt.float8e4
AX = mybir.AxisListType
AF = mybir.ActivationFunctionType
DR = mybir.MatmulPerfMode.DoubleRow

# local tile -> 4-row block of this core's half (block i = rows 4i..4i+3).
# Tile 0 is the block the PAIR core needs as its halo row: for the top
# half (rows 0-15) that's block 3 (row 15), for the bottom half (rows
# 16-31) block 0 (row 16).
TILE_BLOCKS = ([3, 0, 1, 2], [0, 1, 2, 3])

# Strip processing order: boundary query tile (qt 0) gets its last two
# super-chunks early so its AllGather launches ~15us before the stream
# ends.
STRIPS = (
    [(sc, qt) for sc in range(3) for qt in range(N_QT)]
    + [(3, 0), (4, 0)]
    + [(3, qt) for qt in range(1, N_QT)]
    + [(4, qt) for qt in range(1, N_QT)]
)


def _build_program():
    nc = bacc.Bacc(
        "TRN2", target_bir_lowering=False, debug=False, num_devices=N_CORES
    )
    q8d = nc.dram_tensor("q8", [128, NKP, 2, QPC], FP8, kind="ExternalInput").ap()
    db8d = nc.dram_tensor("db8", [128, NKP, 2, N], FP8, kind="ExternalInput").ap()
    q2d = nc.dram_tensor("q2", [128, N_QT], F32, kind="ExternalInput").ap()
    identd = nc.dram_tensor("ident", [128, 2, 128], FP8, kind="ExternalInput").ap()
    s4d = nc.dram_tensor("s4", [128, 4, W], BF16, kind="ExternalInput").ap()
    artd = nc.dram_tensor("art", [NCOL, OROWS], BF16, kind="ExternalInput").ap()
    acd = nc.dram_tensor("ac", [W, OUT_W], BF16, kind="ExternalInput").ap()
    out = nc.dram_tensor("out", [OROWS, OUT_W], F32, kind="ExternalOutput").ap()
    if DEBUG:
        dbg_parts = nc.dram_tensor(
            "dbg_parts", [128, N_HS * 16], F32, kind="ExternalOutput"
        ).ap()
        dbg_oodht = nc.dram_tensor(
            "dbg_oodht", [W, NCOL], BF16, kind="ExternalOutput"
        ).ap()

    with tile.TileContext(nc) as tc:
        with (
            tc.tile_pool(name="static", bufs=1) as sp,
            tc.tile_pool(name="dbh", bufs=6) as dbhp,
            tc.tile_pool(name="db", bufs=9) as dbp,
            tc.tile_pool(name="absv", bufs=4) as avp,
            tc.tile_pool(name="small", bufs=4) as smp,
            tc.tile_pool(name="psum", bufs=4, space="PSUM") as pp,
            tc.tile_pool(name="dram", bufs=1, space="DRAM") as dp,
        ):
            # queries first (needed by the very first matmul)
            q8 = sp.tile([128, NKP, 2, QPC], FP8)
            nc.sync.dma_start(q8[:], q8d[:])

            # super-chunk 0 as 2-col-half tiles for fast startup (the first
            # half of each arrives as two 1000-slot DMAs so the very first
            # quarter's matmuls start ~2us earlier); sc 1..4 as full
            # [128, 2, 4000] tiles per K-pair.
            db0 = {}  # (kp, h) -> tile  (h: slots h*2000..h*2000+2000)
            for h in range(2):
                for kp in range(NKP):
                    db0[(kp, h)] = dbhp.tile(
                        [128, 2, 2000], FP8, tag="dbh", name=f"db0_{kp}_{h}"
                    )
            for qtr in range(4):
                for kp in range(NKP):
                    h, off = divmod(qtr * 1000, 2000)
                    nc.sync.dma_start(
                        db0[(kp, h)][:, :, off : off + 1000],
                        db8d[:, kp, :, qtr * 1000 : (qtr + 1) * 1000],
                    )
            q2_sb = sp.tile([128, N_QT], F32)
            nc.sync.dma_start(q2_sb[:], q2d[:])
            ident = sp.tile([128, 2, 128], FP8)
            nc.sync.dma_start(ident[:], identd[:])
            s4 = sp.tile([128, 4, W], BF16)
            nc.sync.dma_start(s4[:], s4d[:])
            dbt = {}  # sc -> [kp] tiles
            for sc in range(1, N_SC):
                tiles = []
                for kp in range(NKP):
                    t = dbp.tile([128, 2, SC], FP8, tag="db", name=f"db{sc}_{kp}")
                    nc.sync.dma_start(
                        t[:], db8d[:, kp, :, sc * SC : (sc + 1) * SC]
                    )
                    tiles.append(t)
                dbt[sc] = tiles
            # upsample operands, needed only at the very end
            art_sb = sp.tile([NCOL, OROWS], BF16)
            nc.sync.dma_start(art_sb[:], artd[:])
            ac_sb = sp.tile([W, OUT_W], BF16)
            nc.sync.dma_start(ac_sb[:], acd[:])

            # per-query-tile top-8 of each quarter-strip's folded maxima
            parts = [
                sp.tile([128, N_HS * 16], F32, name=f"part{qt}")
                for qt in range(N_QT)
            ]
            oods = [
                sp.tile([128, 1], BF16, name=f"ood{qt}") for qt in range(N_QT)
            ]
            cc_in = dp.tile([128], BF16)
            cc_out = dp.tile([256], BF16)
            # ood_hT[c, j]: j 0..15 own rows (local order), 16..23 the two
            # gathered boundary blocks in rank order; filled incrementally
            # as each query tile finishes
            ood_hT = sp.tile([W, NCOL], BF16)

            def rhs(sc, kp, col, width):
                """db slots [col, col+width) of super-chunk sc, K-pair kp."""
                if sc == 0:
                    h, off = divmod(col, 2000)
                    return db0[(kp, h)][:, :, off : off + width]
                return dbt[sc][kp][:, :, col : col + width]

            def qt_epilogue(qt):
                f8 = smp.tile([128, 8], F32, tag="f8", name=f"f8_{qt}")
                nc.vector.max(f8[:], parts[qt][:])
                # dist_j/3 = sqrt((q2 - 2 t_j) / 9); host passes q2/9
                d3 = smp.tile([128, K_NN], F32, tag="d3", name=f"d3_{qt}")
                nc.scalar.activation(
                    d3[:],
                    f8[:, 0:K_NN],
                    AF.Sqrt,
                    bias=q2_sb[:, qt : qt + 1],
                    scale=-2.0 / 9.0,
                )
                with nc.allow_low_precision(
                    reason="3-element sum rounded to bf16 for the ood "
                    "exchange; ~0.2% on a 2% tolerance"
                ):
                    nc.vector.reduce_sum(oods[qt][:], d3[:], axis=AX.X)
                if qt == 0:
                    # boundary block: gather it across the pair ASAP so
                    # the ~15us collective hides under remaining work
                    nc.sync.dma_start(cc_in[:], oods[0][:])
                    nc.gpsimd.collective_compute(
                        "AllGather",
                        mybir.AluOpType.bypass,
                        replica_groups=[[0, 1], [2, 3], [4, 5], [6, 7]],
                        ins=[cc_in.opt()],
                        outs=[cc_out.opt()],
                    )
                    nc.sync.dma_start(
                        ood_hT[:, 16:NCOL],
                        cc_out.rearrange("(b r c) -> c (b r)", b=2, c=W),
                    )
                # the [128,1] -> [32,4] transpose into ood_hT is deferred
                # to after the strips loop: its selector matmuls would
                # otherwise block the in-order PE queue on this reduce

            # deferred per-half-strip work: (u_tile, absv_tile, part_ap)
            pending = []

            def drain_one():
                u, absv, part_ap = pending.pop(0)
                # u += |v| closes the folded accumulation group (exact
                # max up to fp8 rounding of |v|: u + |v| = max(t0, t1)).
                # DoubleRow at 0.5 cyc/col: stationary k-tile0 = identity,
                # k-tile1 = zeros; absv's k-tile1 is memset to 0 once.
                nc.tensor.matmul(
                    u[:, 0, 0:500],
                    ident[:],
                    absv[:, :, 0:500],
                    start=False,
                    stop=True,
                    perf_mode=DR,
                )
                nc.vector.max(part_ap, u[:, 0, 0:500])

            for si, (sc, qt) in enumerate(STRIPS):
                lhsT = [
                    q8[:, kp, :, qt * 128 : (qt + 1) * 128] for kp in range(NKP)
                ]
                for qr in range(4):
                    g0 = qr * 1000          # in-chunk slot base
                    # a-slots [g0, g0+500) -> u bank,
                    # b-slots [g0+500, g0+1000) -> v bank; 1-bank tiles in
                    # a 4-deep rotation so the deferred id-add + max8 never
                    # stall the PE on PSUM reuse
                    u_ps = pp.tile([128, 1, 512], F32, tag="u", name="u", bufs=4)
                    v_ps = pp.tile([128, 1, 512], F32, tag="v", name="v", bufs=4)
                    for kp in range(NKP):
                        nc.tensor.matmul(
                            u_ps[:, 0, 0:500],
                            lhsT[kp],
                            rhs(sc, kp, g0, 500),
                            start=(kp == 0),
                            stop=False,
                            perf_mode=DR,
                        )
                        nc.tensor.matmul(
                            v_ps[:, 0, 0:500],
                            lhsT[kp],
                            rhs(sc, kp, g0 + 500, 500),
                            start=(kp == 0),
                            stop=(kp == NKP - 1),
                            perf_mode=DR,
                        )
                    # ScalarE: |v| -> SBUF fp8 (k-tile1 zeroed once so the
                    # DoubleRow id-add's second lane contributes nothing)
                    absv = avp.tile([128, 2, 500], FP8, tag="absv", name="absv")
                    if si * 4 + qr < 4:
                        nc.gpsimd.memset(absv[:, 1, 0:500], 0.0)
                    nc.scalar.activation(
                        absv[:, 0, 0:500], v_ps[:, 0, 0:500], AF.Abs
                    )
                    pending.append(
                        (u_ps, absv, parts[qt][:, (sc * 4 + qr) * 8 :][:, 0:8])
                    )
                    # drain the 2-quarters-old id-add + max8 here: by then
                    # its |v| activation has long finished, so neither PE
                    # nor DVE stalls on the ScalarE chain
                    while len(pending) > 3:
                        drain_one()

                is_qt_last = (sc, qt) in ((4, 0), (4, 1), (4, 2), (4, 3))
                if is_qt_last:
                    while pending:
                        drain_one()
                    qt_epilogue(qt)

            # own blocks into the upsample operand: the PE transposes each
            # [128,1] ood vector into [32,4] with 4 per-row-block selector
            # matmuls (S4[:,b,:].T @ ood picks partitions b*32..b*32+31),
            # then ScalarE drops it into ood_hT -- much lower latency than
            # a DMA round-trip through DRAM, and emitted here so only the
            # last query tile's transpose is on the critical path
            for qt in range(N_QT):
                oht_ps = pp.tile([W, 4], F32, tag="u", name=f"oht{qt}", bufs=4)
                for blk in range(4):
                    nc.tensor.matmul(
                        oht_ps[:, blk : blk + 1],
                        s4[:, blk, :],
                        oods[qt][:],
                        start=True,
                        stop=True,
                    )
                nc.scalar.activation(
                    ood_hT[:, qt * 4 : (qt + 1) * 4], oht_ps[:], AF.Copy
                )

            if DEBUG:
                nc.sync.dma_start(dbg_parts[:], parts[0][:])
                nc.sync.dma_start(dbg_oodht[:], ood_hT[:])

            # P1[j, ow] = sum_c ood_hT[c, j] * A_c[c, ow]
            p1 = pp.tile([NCOL, OUT_W], F32, tag="v", name="p1", bufs=4)
            nc.tensor.matmul(p1[:], ood_hT[:], ac_sb[:], start=True, stop=True)
            p1_sb = sp.tile([NCOL, OUT_W], BF16)
            nc.scalar.activation(p1_sb[:], p1[:], AF.Copy)
            # out[oi, ow] = sum_j art[j, oi] * P1[j, ow]
            for m in range(2):
                p2 = pp.tile([128, OUT_W], F32, tag="v", name=f"p2_{m}", bufs=4)
                nc.tensor.matmul(
                    p2[:],
                    art_sb[:, m * 128 : (m + 1) * 128],
                    p1_sb[:],
                    start=True,
                    stop=True,
                )
                o_sb = smp.tile([128, OUT_W], F32, tag="osb", name=f"osb{m}")
                nc.scalar.activation(o_sb[:], p2[:], AF.Copy)
                nc.sync.dma_start(out[m * 128 : (m + 1) * 128, :], o_sb[:])

    nc.compile()
    return nc


def _bilinear_matrix(out_size: int, in_size: int) -> np.ndarray:
    """Half-pixel (align_corners=False) bilinear interpolation matrix
    [out_size, in_size]; edge-clamped, equivalent to jax.image.resize
    'bilinear' for integer upsampling."""
    A = np.zeros((out_size, in_size), dtype=np.float64)
    scale = in_size / out_size
    for i in range(out_size):
        s = (i + 0.5) * scale - 0.5
        j0 = int(np.floor(s))
        w = s - j0
        A[i, min(max(j0, 0), in_size - 1)] += 1.0 - w
        A[i, min(max(j0 + 1, 0), in_size - 1)] += w
    return A.astype(np.float32)


_NC_CACHE = None


def _get_nc():
    global _NC_CACHE
    if _NC_CACHE is None:
        _NC_CACHE = _build_program()
    return _NC_CACHE


def _slot_pack(database: np.ndarray):
    """Fold all column pairs: per 1000-col quarter-strip the slot layout
    is [500 a=(x0+x1)/2 | 500 b=(x0-x1)/2] over 764 dims, with the
    matching xh = -(||x0||^2 +- ||x1||^2)/4 terms (over all 768 dims,
    scaled 1/XS) as a 4-level fp8 split in rows 764..767."""
    h = 0.5 * np.einsum("nd,nd->n", database, database)    # ||x||^2/2
    dbX = np.empty((N, 768), dtype=np.float32)             # slot-major
    xhX = np.empty(N, dtype=np.float32)
    for g in range(N // 1000):
        base = g * 1000
        p0 = database[base : base + 1000 : 2, :DX]
        p1 = database[base + 1 : base + 1000 : 2, :DX]
        h0 = h[base : base + 1000 : 2]
        h1 = h[base + 1 : base + 1000 : 2]
        dbX[base : base + 500, :DX] = 0.5 * (p0 + p1)
        dbX[base + 500 : base + 1000, :DX] = 0.5 * (p0 - p1)
        xhX[base : base + 500] = -0.5 * (h0 + h1)
        xhX[base + 500 : base + 1000] = -0.5 * (h0 - h1)
    # 4-level fp8 split of xh/XS into the 4 spare contraction rows
    r = (xhX / XS).astype(np.float32)
    for lv in range(4):
        q = r.astype(ml_dtypes.float8_e4m3).astype(np.float32)
        dbX[:, DX + lv] = q
        r = r - q
    return dbX


def make_in_maps(embeddings: np.ndarray, database: np.ndarray):
    embeddings = np.asarray(embeddings, dtype=np.float32)
    database = np.asarray(database, dtype=np.float32)

    dbX = _slot_pack(database)
    # contraction-pair layout: db8[p, kp, i, n] = dbX[n, kp*256+i*128+p]
    db8 = np.ascontiguousarray(
        dbX.T.reshape(NKP, 2, 128, N).transpose(2, 0, 1, 3)
    ).astype(ml_dtypes.float8_e4m3)

    q_all = embeddings.transpose(0, 2, 3, 1).reshape(B, H * W, D)
    Ac = _bilinear_matrix(OUT_W, W)                      # [512, 32]
    Ar = _bilinear_matrix(OUT_H, H)                      # [512, 32]
    # DoubleRow identity: k-tile0 = I (adds |v|), k-tile1 = 0
    ident = np.zeros((128, 2, 128), dtype=np.float32)
    ident[:, 0, :] = np.eye(128, dtype=np.float32)
    ident = ident.astype(ml_dtypes.float8_e4m3)
    # s4[q, b, c] = 1 iff q == b*32+c: per-row-block selectors that let the
    # PE transpose a [128,1] ood vector into [32,4] map layout
    s4 = np.eye(128, dtype=np.float32).reshape(128, 4, W).astype(
        ml_dtypes.bfloat16
    )
    # the two gathered blocks in cc_out rank order: pair-core tile 0 rows
    cc_rows = [12, 13, 14, 15, 16, 17, 18, 19]

    in_maps = []
    for c in range(N_CORES):
        b, half = divmod(c, 2)
        blocks = TILE_BLOCKS[half]
        own_rows = [16 * half + 4 * blk + r for blk in blocks for r in range(4)]

        # queries in local-tile order; the 4 spare rows carry the xh
        # stationary scale XS
        q = np.concatenate(
            [
                q_all[b, (16 * half + 4 * blk) * W : (16 * half + 4 * blk + 4) * W]
                for blk in blocks
            ]
        )                                                # [512, 768]
        qX = q.copy()
        qX[:, DX:] = XS
        q8 = np.ascontiguousarray(
            qX.T.reshape(NKP, 2, 128, QPC).transpose(2, 0, 1, 3)
        ).astype(ml_dtypes.float8_e4m3)                  # [128, 3, 2, 512]
        q2 = np.einsum("qd,qd->q", q, q) / 9.0
        q2 = np.ascontiguousarray(q2.reshape(N_QT, 128).T.astype(np.float32))

        # interpolation rows matching ood_hT's column order
        Arh = Ar[half * OROWS : (half + 1) * OROWS]      # [256, 32]
        art = np.zeros((NCOL, OROWS), dtype=np.float32)
        for j, row in enumerate(own_rows):
            art[j] = Arh[:, row]
        for j, row in enumerate(cc_rows):
            if row not in own_rows:
                art[16 + j] = Arh[:, row]
        in_maps.append(
            {
                "db8": db8,
                "q8": q8,
                "q2": q2,
                "ident": ident,
                "s4": s4,
                "art": art.astype(ml_dtypes.bfloat16),
                "ac": np.ascontiguousarray(Ac.T).astype(ml_dtypes.bfloat16),
            }
        )
    return in_maps


def run_device(in_maps, **kwargs):
    nc = _get_nc()
    return bass_utils.run_bass_kernel_spmd(
        nc, in_maps, core_ids=list(range(N_CORES)), **kwargs
    )


def kernel(embeddings, database, k, out_h, out_w):
    assert int(k) == K_NN and int(out_h) == OUT_H and int(out_w) == OUT_W
    in_maps = make_in_maps(np.asarray(embeddings), np.asarray(database))
    res = run_device(in_maps)
    out = np.empty((B, 1, OUT_H, OUT_W), dtype=np.float32)
    for c in range(N_CORES):
        b, half = divmod(c, 2)
        out[b, 0, half * OROWS : (half + 1) * OROWS] = res.results[c]["out"]
    return out


# revision 78
# speedup vs baseline: 1.0007x; 1.0007x over previous
"""Distributed kNN OOD-score kernel for 8 Trainium2 NeuronCores.

Problem: for each of 4*32*32 query vectors (D=768), find the 3 nearest
database vectors (N=20000, squared-L2), average the 3 distances, and
bilinearly upsample the resulting [4,32,32] map to [4,1,512,512].

Sharding: queries are data-parallel. Each core owns half of one batch
image (16 of 32 query rows = 512 queries); the database is replicated
and streamed through SBUF in fp8 (e4m3). The one halo row each core
needs for the 16x bilinear upsample is exchanged with its pair core via
a tiny AllGather, scheduled early (boundary query tile first, its last
two super-chunks processed ahead of the other tiles) so its ~15us
latency hides under the remaining compute stream.

Per-core device program (v4, fp8 DoubleRow + full PE-side max-fold):
  - every database column PAIR (x0, x1) is pre-folded on the host into
    a=(x0+x1)/2 and b=(x0-x1)/2 streams. The device computes
        u = q.a + xh_a   and   v = q.b + xh_b
    (so u,v = (t0+-t1)/2 for scores t = q.x - ||x||^2/2), ScalarE takes
    |v| to SBUF bf16, and TensorE adds it onto u with a bf16 identity
    matmul: u + |v| = max(t0, t1) exactly. VectorE then scans only
    1000 folded values per 2000-col half-strip.
  - the cross products keep 764 of 768 dims; the 4 freed contraction
    rows carry xh_* as a 4-level fp8 split (x4 stationary scale baked
    into constant query rows), so scoring is EXACTLY 3 fp8 e4m3
    DoubleRow matmuls per bank (0.5 cyc/col each). ||q||^2 and ||x||^2
    stay exact over all 768 dims; the dropped cross terms add ~4 rms
    on d^2 ~ 1536 (top-3 spacing ~20) -- well inside the tolerance.
  - per query-tile: max8 over the 10 strip top-8s -> top-3, then
    mean distance = reduce_sum of sqrt((q^2 - 2t)/9); ood values are
    transposed into map layout with 4 tiny selector matmuls on the PE
    (no DRAM round-trip).
  - pair AllGather of the boundary tile's 128 ood values (bf16).
  - 16x bilinear upsample = two small bf16 matmuls (interp weights are
    odd/32 fractions - exact in bf16; verified vs jax.image.resize).
"""

import sys

if "/opt/trn_rl_repo" not in sys.path:
    sys.path.insert(0, "/opt/trn_rl_repo")

import numpy as np
import ml_dtypes

import concourse.bass as bass
import concourse.bacc as bacc
import concourse.mybir as mybir
import concourse.tile as tile
from concourse import bass_utils

# Problem shape (hardcoded per contract).
B, D, H, W = 4, 768, 32, 32
N = 20000
K_NN = 3
OUT_H = OUT_W = 512
N_CORES = 8

SC = 4000            # db columns per super-chunk
N_SC = N // SC       # 5
N_HS = 2 * N_SC      # 2000-col half-strips per query tile
QPC = 512            # queries scored per core (16 rows)
N_QT = QPC // 128    # 4
NKP = 3              # K pairs: 764 data dims + 4 xh rows = 3 * (2*128)
DX = 764             # cross-term dims (768 minus 4 freed for xh rows)
OROWS = 256          # output rows per core
NCOL = 24            # ood columns entering the upsample (16 own + 2x4 gathered)
XS = 4.0             # xh scale, baked into constant query rows (fp8 range fit)
DEBUG = False        # adds intermediate-tensor outputs for debugging

F32 = mybir.dt.float32
BF16 = mybir.dt.bfloat16
FP8 = mybir.dt.float8e4
AX = mybir.AxisListType
AF = mybir.ActivationFunctionType
DR = mybir.MatmulPerfMode.DoubleRow

# local tile -> 4-row block of this core's half (block i = rows 4i..4i+3).
# Tile 0 is the block the PAIR core needs as its halo row: for the top
# half (rows 0-15) that's block 3 (row 15), for the bottom half (rows
# 16-31) block 0 (row 16).
TILE_BLOCKS = ([3, 0, 1, 2], [0, 1, 2, 3])

# Strip processing order: boundary query tile (qt 0) gets its last two
# super-chunks early so its AllGather launches ~15us before the stream
# ends.
STRIPS = (
    [(sc, qt) for sc in range(3) for qt in range(N_QT)]
    + [(3, 0), (4, 0)]
    + [(3, qt) for qt in range(1, N_QT)]
    + [(4, qt) for qt in range(1, N_QT)]
)


def _build_program():
    nc = bacc.Bacc(
        "TRN2", target_bir_lowering=False, debug=False, num_devices=N_CORES
    )
    q8d = nc.dram_tensor("q8", [128, NKP, 2, QPC], FP8, kind="ExternalInput").ap()
    db8d = nc.dram_tensor("db8", [128, NKP, 2, N], FP8, kind="ExternalInput").ap()
    q2d = nc.dram_tensor("q2", [128, N_QT], F32, kind="ExternalInput").ap()
    identd = nc.dram_tensor("ident", [128, 2, 128], FP8, kind="ExternalInput").ap()
    s4d = nc.dram_tensor("s4", [128, 4, W], BF16, kind="ExternalInput").ap()
    artd = nc.dram_tensor("art", [NCOL, OROWS], BF16, kind="ExternalInput").ap()
    acd = nc.dram_tensor("ac", [W, OUT_W], BF16, kind="ExternalInput").ap()
    out = nc.dram_tensor("out", [OROWS, OUT_W], F32, kind="ExternalOutput").ap()
    if DEBUG:
        dbg_parts = nc.dram_tensor(
            "dbg_parts", [128, N_HS * 16], F32, kind="ExternalOutput"
        ).ap()
        dbg_oodht = nc.dram_tensor(
            "dbg_oodht", [W, NCOL], BF16, kind="ExternalOutput"
        ).ap()

    with tile.TileContext(nc) as tc:
        with (
            tc.tile_pool(name="static", bufs=1) as sp,
            tc.tile_pool(name="dbh", bufs=6) as dbhp,
            tc.tile_pool(name="db", bufs=9) as dbp,
            tc.tile_pool(name="absv", bufs=5) as avp,
            tc.tile_pool(name="small", bufs=4) as smp,
            tc.tile_pool(name="psum", bufs=4, space="PSUM") as pp,
            tc.tile_pool(name="dram", bufs=1, space="DRAM") as dp,
        ):
            # queries first (needed by the very first matmul)
            q8 = sp.tile([128, NKP, 2, QPC], FP8)
            nc.sync.dma_start(q8[:], q8d[:])

            # super-chunk 0 as 2-col-half tiles for fast startup (the first
            # half of each arrives as two 1000-slot DMAs so the very first
            # quarter's matmuls start ~2us earlier); sc 1..4 as full
            # [128, 2, 4000] tiles per K-pair.
            db0 = {}  # (kp, h) -> tile  (h: slots h*2000..h*2000+2000)
            for h in range(2):
                for kp in range(NKP):
                    db0[(kp, h)] = dbhp.tile(
                        [128, 2, 2000], FP8, tag="dbh", name=f"db0_{kp}_{h}"
                    )
            for qtr in range(4):
                for kp in range(NKP):
                    h, off = divmod(qtr * 1000, 2000)
                    nc.sync.dma_start(
                        db0[(kp, h)][:, :, off : off + 1000],
                        db8d[:, kp, :, qtr * 1000 : (qtr + 1) * 1000],
                    )
            q2_sb = sp.tile([128, N_QT], F32)
            nc.sync.dma_start(q2_sb[:], q2d[:])
            ident = sp.tile([128, 2, 128], FP8)
            nc.sync.dma_start(ident[:], identd[:])
            s4 = sp.tile([128, 4, W], BF16)
            nc.sync.dma_start(s4[:], s4d[:])
            dbt = {}  # sc -> [kp] tiles
            for sc in range(1, N_SC):
                tiles = []
                for kp in range(NKP):
                    t = dbp.tile([128, 2, SC], FP8, tag="db", name=f"db{sc}_{kp}")
                    nc.sync.dma_start(
                        t[:], db8d[:, kp, :, sc * SC : (sc + 1) * SC]
                    )
                    tiles.append(t)
                dbt[sc] = tiles
            # upsample operands, needed only at the very end
            art_sb = sp.tile([NCOL, OROWS], BF16)
            nc.sync.dma_start(art_sb[:], artd[:])
            ac_sb = sp.tile([W, OUT_W], BF16)
            nc.sync.dma_start(ac_sb[:], acd[:])

            # per-query-tile top-8 of each quarter-strip's folded maxima
            parts = [
                sp.tile([128, N_HS * 16], F32, name=f"part{qt}")
                for qt in range(N_QT)
            ]
            oods = [
                sp.tile([128, 1], BF16, name=f"ood{qt}") for qt in range(N_QT)
            ]
            cc_in = dp.tile([128], BF16)
            cc_out = dp.tile([256], BF16)
            # ood_hT[c, j]: j 0..15 own rows (local order), 16..23 the two
            # gathered boundary blocks in rank order; filled incrementally
            # as each query tile finishes
            ood_hT = sp.tile([W, NCOL], BF16)

            def rhs(sc, kp, col, width):
                """db slots [col, col+width) of super-chunk sc, K-pair kp."""
                if sc == 0:
                    h, off = divmod(col, 2000)
                    return db0[(kp, h)][:, :, off : off + width]
                return dbt[sc][kp][:, :, col : col + width]

            def qt_epilogue(qt):
                f8 = smp.tile([128, 8], F32, tag="f8", name=f"f8_{qt}")
                nc.vector.max(f8[:], parts[qt][:])
                # dist_j/3 = sqrt((q2 - 2 t_j) / 9); host passes q2/9
                d3 = smp.tile([128, K_NN], F32, tag="d3", name=f"d3_{qt}")
                nc.scalar.activation(
                    d3[:],
                    f8[:, 0:K_NN],
                    AF.Sqrt,
                    bias=q2_sb[:, qt : qt + 1],
                    scale=-2.0 / 9.0,
                )
                with nc.allow_low_precision(
                    reason="3-element sum rounded to bf16 for the ood "
                    "exchange; ~0.2% on a 2% tolerance"
                ):
                    nc.vector.reduce_sum(oods[qt][:], d3[:], axis=AX.X)
                if qt == 0:
                    # boundary block: gather it across the pair ASAP so
                    # the ~15us collective hides under remaining work
                    nc.sync.dma_start(cc_in[:], oods[0][:])
                    nc.gpsimd.collective_compute(
                        "AllGather",
                        mybir.AluOpType.bypass,
                        replica_groups=[[0, 1], [2, 3], [4, 5], [6, 7]],
                        ins=[cc_in.opt()],
                        outs=[cc_out.opt()],
                    )
                    nc.sync.dma_start(
                        ood_hT[:, 16:NCOL],
                        cc_out.rearrange("(b r c) -> c (b r)", b=2, c=W),
                    )
                # the [128,1] -> [32,4] transpose into ood_hT is deferred
                # to after the strips loop: its selector matmuls would
                # otherwise block the in-order PE queue on this reduce

            # deferred per-half-strip work: (u_tile, absv_tile, part_ap)
            pending = []

            def drain_one():
                u, absv, part_ap = pending.pop(0)
                # u += |v| closes the folded accumulation group (exact
                # max up to fp8 rounding of |v|: u + |v| = max(t0, t1)).
                # DoubleRow at 0.5 cyc/col: stationary k-tile0 = identity,
                # k-tile1 = zeros; absv's k-tile1 is memset to 0 once.
                nc.tensor.matmul(
                    u[:, 0, 0:500],
                    ident[:],
                    absv[:, :, 0:500],
                    start=False,
                    stop=True,
                    perf_mode=DR,
                )
                nc.vector.max(part_ap, u[:, 0, 0:500])

            for si, (sc, qt) in enumerate(STRIPS):
                lhsT = [
                    q8[:, kp, :, qt * 128 : (qt + 1) * 128] for kp in range(NKP)
                ]
                for qr in range(4):
                    g0 = qr * 1000          # in-chunk slot base
                    # a-slots [g0, g0+500) -> u bank,
                    # b-slots [g0+500, g0+1000) -> v bank; 1-bank tiles in
                    # a 4-deep rotation so the deferred id-add + max8 never
                    # stall the PE on PSUM reuse
                    u_ps = pp.tile([128, 1, 512], F32, tag="u", name="u", bufs=4)
                    v_ps = pp.tile([128, 1, 512], F32, tag="v", name="v", bufs=4)
                    for kp in range(NKP):
                        nc.tensor.matmul(
                            u_ps[:, 0, 0:500],
                            lhsT[kp],
                            rhs(sc, kp, g0, 500),
                            start=(kp == 0),
                            stop=False,
                            perf_mode=DR,
                        )
                        nc.tensor.matmul(
                            v_ps[:, 0, 0:500],
                            lhsT[kp],
                            rhs(sc, kp, g0 + 500, 500),
                            start=(kp == 0),
                            stop=(kp == NKP - 1),
                            perf_mode=DR,
                        )
                    # ScalarE: |v| -> SBUF fp8 (k-tile1 zeroed once so the
                    # DoubleRow id-add's second lane contributes nothing)
                    absv = avp.tile([128, 2, 500], FP8, tag="absv", name="absv")
                    if si * 4 + qr < 5:
                        nc.gpsimd.memset(absv[:, 1, 0:500], 0.0)
                    nc.scalar.activation(
                        absv[:, 0, 0:500], v_ps[:, 0, 0:500], AF.Abs
                    )
                    pending.append(
                        (u_ps, absv, parts[qt][:, (sc * 4 + qr) * 8 :][:, 0:8])
                    )
                    # drain the 2-quarters-old id-add + max8 here: by then
                    # its |v| activation has long finished, so neither PE
                    # nor DVE stalls on the ScalarE chain
                    while len(pending) > 3:
                        drain_one()

                is_qt_last = (sc, qt) in ((4, 0), (4, 1), (4, 2), (4, 3))
                if is_qt_last:
                    while pending:
                        drain_one()
                    qt_epilogue(qt)

            # own blocks into the upsample operand: the PE transposes each
            # [128,1] ood vector into [32,4] with 4 per-row-block selector
            # matmuls (S4[:,b,:].T @ ood picks partitions b*32..b*32+31),
            # then ScalarE drops it into ood_hT -- much lower latency than
            # a DMA round-trip through DRAM, and emitted here so only the
            # last query tile's transpose is on the critical path
            for qt in range(N_QT):
                oht_ps = pp.tile([W, 4], F32, tag="u", name=f"oht{qt}", bufs=4)
                for blk in range(4):
                    nc.tensor.matmul(
                        oht_ps[:, blk : blk + 1],
                        s4[:, blk, :],
                        oods[qt][:],
                        start=True,
                        stop=True,
                    )
                nc.scalar.activation(
                    ood_hT[:, qt * 4 : (qt + 1) * 4], oht_ps[:], AF.Copy
                )

            if DEBUG:
                nc.sync.dma_start(dbg_parts[:], parts[0][:])
                nc.sync.dma_start(dbg_oodht[:], ood_hT[:])

            # P1[j, ow] = sum_c ood_hT[c, j] * A_c[c, ow]
            p1 = pp.tile([NCOL, OUT_W], F32, tag="v", name="p1", bufs=4)
            nc.tensor.matmul(p1[:], ood_hT[:], ac_sb[:], start=True, stop=True)
            p1_sb = sp.tile([NCOL, OUT_W], BF16)
            nc.scalar.activation(p1_sb[:], p1[:], AF.Copy)
            # out[oi, ow] = sum_j art[j, oi] * P1[j, ow]
            for m in range(2):
                p2 = pp.tile([128, OUT_W], F32, tag="v", name=f"p2_{m}", bufs=4)
                nc.tensor.matmul(
                    p2[:],
                    art_sb[:, m * 128 : (m + 1) * 128],
                    p1_sb[:],
                    start=True,
                    stop=True,
                )
                o_sb = smp.tile([128, OUT_W], F32, tag="osb", name=f"osb{m}")
                nc.scalar.activation(o_sb[:], p2[:], AF.Copy)
                nc.sync.dma_start(out[m * 128 : (m + 1) * 128, :], o_sb[:])

    nc.compile()
    return nc


def _bilinear_matrix(out_size: int, in_size: int) -> np.ndarray:
    """Half-pixel (align_corners=False) bilinear interpolation matrix
    [out_size, in_size]; edge-clamped, equivalent to jax.image.resize
    'bilinear' for integer upsampling."""
    A = np.zeros((out_size, in_size), dtype=np.float64)
    scale = in_size / out_size
    for i in range(out_size):
        s = (i + 0.5) * scale - 0.5
        j0 = int(np.floor(s))
        w = s - j0
        A[i, min(max(j0, 0), in_size - 1)] += 1.0 - w
        A[i, min(max(j0 + 1, 0), in_size - 1)] += w
    return A.astype(np.float32)


_NC_CACHE = None


def _get_nc():
    global _NC_CACHE
    if _NC_CACHE is None:
        _NC_CACHE = _build_program()
    return _NC_CACHE


def _slot_pack(database: np.ndarray):
    """Fold all column pairs: per 1000-col quarter-strip the slot layout
    is [500 a=(x0+x1)/2 | 500 b=(x0-x1)/2] over 764 dims, with the
    matching xh = -(||x0||^2 +- ||x1||^2)/4 terms (over all 768 dims,
    scaled 1/XS) as a 4-level fp8 split in rows 764..767."""
    h = 0.5 * np.einsum("nd,nd->n", database, database)    # ||x||^2/2
    dbX = np.empty((N, 768), dtype=np.float32)             # slot-major
    xhX = np.empty(N, dtype=np.float32)
    for g in range(N // 1000):
        base = g * 1000
        p0 = database[base : base + 1000 : 2, :DX]
        p1 = database[base + 1 : base + 1000 : 2, :DX]
        h0 = h[base : base + 1000 : 2]
        h1 = h[base + 1 : base + 1000 : 2]
        dbX[base : base + 500, :DX] = 0.5 * (p0 + p1)
        dbX[base + 500 : base + 1000, :DX] = 0.5 * (p0 - p1)
        xhX[base : base + 500] = -0.5 * (h0 + h1)
        xhX[base + 500 : base + 1000] = -0.5 * (h0 - h1)
    # 4-level fp8 split of xh/XS into the 4 spare contraction rows
    r = (xhX / XS).astype(np.float32)
    for lv in range(4):
        q = r.astype(ml_dtypes.float8_e4m3).astype(np.float32)
        dbX[:, DX + lv] = q
        r = r - q
    return dbX


def make_in_maps(embeddings: np.ndarray, database: np.ndarray):
    embeddings = np.asarray(embeddings, dtype=np.float32)
    database = np.asarray(database, dtype=np.float32)

    dbX = _slot_pack(database)
    # contraction-pair layout: db8[p, kp, i, n] = dbX[n, kp*256+i*128+p]
    db8 = np.ascontiguousarray(
        dbX.T.reshape(NKP, 2, 128, N).transpose(2, 0, 1, 3)
    ).astype(ml_dtypes.float8_e4m3)

    q_all = embeddings.transpose(0, 2, 3, 1).reshape(B, H * W, D)
    Ac = _bilinear_matrix(OUT_W, W)                      # [512, 32]
    Ar = _bilinear_matrix(OUT_H, H)                      # [512, 32]
    # DoubleRow identity: k-tile0 = I (adds |v|), k-tile1 = 0
    ident = np.zeros((128, 2, 128), dtype=np.float32)
    ident[:, 0, :] = np.eye(128, dtype=np.float32)
    ident = ident.astype(ml_dtypes.float8_e4m3)
    # s4[q, b, c] = 1 iff q == b*32+c: per-row-block selectors that let the
    # PE transpose a [128,1] ood vector into [32,4] map layout
    s4 = np.eye(128, dtype=np.float32).reshape(128, 4, W).astype(
        ml_dtypes.bfloat16
    )
    # the two gathered blocks in cc_out rank order: pair-core tile 0 rows
    cc_rows = [12, 13, 14, 15, 16, 17, 18, 19]

    in_maps = []
    for c in range(N_CORES):
        b, half = divmod(c, 2)
        blocks = TILE_BLOCKS[half]
        own_rows = [16 * half + 4 * blk + r for blk in blocks for r in range(4)]

        # queries in local-tile order; the 4 spare rows carry the xh
        # stationary scale XS
        q = np.concatenate(
            [
                q_all[b, (16 * half + 4 * blk) * W : (16 * half + 4 * blk + 4) * W]
                for blk in blocks
            ]
        )                                                # [512, 768]
        qX = q.copy()
        qX[:, DX:] = XS
        q8 = np.ascontiguousarray(
            qX.T.reshape(NKP, 2, 128, QPC).transpose(2, 0, 1, 3)
        ).astype(ml_dtypes.float8_e4m3)                  # [128, 3, 2, 512]
        q2 = np.einsum("qd,qd->q", q, q) / 9.0
        q2 = np.ascontiguousarray(q2.reshape(N_QT, 128).T.astype(np.float32))

        # interpolation rows matching ood_hT's column order
        Arh = Ar[half * OROWS : (half + 1) * OROWS]      # [256, 32]
        art = np.zeros((NCOL, OROWS), dtype=np.float32)
        for j, row in enumerate(own_rows):
            art[j] = Arh[:, row]
        for j, row in enumerate(cc_rows):
            if row not in own_rows:
                art[16 + j] = Arh[:, row]
        in_maps.append(
            {
                "db8": db8,
                "q8": q8,
                "q2": q2,
                "ident": ident,
                "s4": s4,
                "art": art.astype(ml_dtypes.bfloat16),
                "ac": np.ascontiguousarray(Ac.T).astype(ml_dtypes.bfloat16),
            }
        )
    return in_maps


def run_device(in_maps, **kwargs):
    nc = _get_nc()
    return bass_utils.run_bass_kernel_spmd(
        nc, in_maps, core_ids=list(range(N_CORES)), **kwargs
    )


def kernel(embeddings, database, k, out_h, out_w):
    assert int(k) == K_NN and int(out_h) == OUT_H and int(out_w) == OUT_W
    in_maps = make_in_maps(np.asarray(embeddings), np.asarray(database))
    res = run_device(in_maps)
    out = np.empty((B, 1, OUT_H, OUT_W), dtype=np.float32)
    for c in range(N_CORES):
        b, half = divmod(c, 2)
        out[b, 0, half * OROWS : (half + 1) * OROWS] = res.results[c]["out"]
    return out
